# revision 3
# baseline (speedup 1.0000x reference)
"""Trainium2 Bass kernel for nn_NodeUpdateV2 (GNN message passing).

Sharding: nodes split into 8 contiguous ranges (1 per core); edges
partitioned by destination node so scatter-mean is core-local. Per
supertile (group of dst tiles) a local table of unique source-node
embeddings is materialized host-side (the halo exchange); the per-edge
gather from that table runs on device via dma_gather. Messages are
computed with PE matmuls; scatter-mean is a one-hot segment matmul with
1/cnt folded in host-side; the node update is fused per dst tile.

Self-contained: hardcodes the problem shapes from the spec.
"""
import math
import numpy as np
from contextlib import ExitStack

import concourse.tile as tile
from concourse import bass, bacc, mybir, library_config
from concourse.bass_utils import run_bass_kernel_spmd
from concourse.masks import make_identity

P = 128  # partitions / dst tile size / msg dim

FULL_DIMS = dict(
    N=200000, E_ATTR=600000, E_REL=800000,
    NODE_DIM=128, EDGE_DIM=64, MSG_DIM=128, OUT_DIM=128, N_REL=8,
    NCORES=8, NPC=25000, ST=16,  # ST = dst tiles per supertile
)


def _ceil(a, b):
    return -(-a // b)


def prep(inputs, dims):
    """Host-side sharding. Returns (plan, per_core_maps).

    plan is identical for all cores (program structure); per-core maps
    hold the data tensors.
    """
    d = dims
    NC, NPC, ST = d["NCORES"], d["NPC"], d["ST"]
    T = _ceil(NPC, P)          # dst tiles per core
    NPC_PAD = T * P
    S = _ceil(T, ST)           # supertiles
    st_lo = [s * ST for s in range(S)]
    st_hi = [min((s + 1) * ST, T) for s in range(S)]

    node_emb = np.asarray(inputs["node_emb"], np.float32)
    edge_emb = np.asarray(inputs["edge_emb"], np.float32)
    dei = np.asarray(inputs["data_edge_index"], np.int64)
    rei = np.asarray(inputs["rel_edge_index"], np.int64)
    rtype = np.asarray(inputs["rel_edge_type"], np.int64)
    is_unit = np.asarray(inputs["is_unit"], bool)

    src_a, dst_a = dei[0], dei[1]
    src_r, dst_r = rei[0], rei[1]
    N, NREL = d["N"], d["N_REL"]

    # mean denominators (global == per-core since every edge lands on one core)
    cnt_a = np.bincount(dst_a, minlength=N).astype(np.float32)
    sca_e = 1.0 / np.maximum(cnt_a, 1.0)
    cnt_r = np.bincount(dst_r * NREL + rtype, minlength=N * NREL).astype(np.float32)
    scr_e = 1.0 / np.maximum(cnt_r, 1.0)

    core_a = (dst_a // NPC).astype(np.int32)
    ldst_a = (dst_a % NPC).astype(np.int64)
    t_a = (ldst_a // P).astype(np.int32)
    off_a = (ldst_a % P).astype(np.float32)
    core_r = (dst_r // NPC).astype(np.int32)
    ldst_r = (dst_r % NPC).astype(np.int64)
    t_r = (ldst_r // P).astype(np.int32)
    off_r = (ldst_r % P).astype(np.float32)

    # ---- attr chunk caps: per tile, max edge count over cores ----
    cntAct = np.zeros((NC, T), np.int64)
    np.add.at(cntAct, (core_a, t_a), 1)
    capA = cntAct.max(axis=0)                      # [T]
    chunksA = np.maximum(_ceil(capA, P), 1)        # >=1 chunk per tile
    slotA_base = np.concatenate([[0], np.cumsum(chunksA)]).astype(np.int64)
    SA = int(slotA_base[-1])                       # total attr slots (128 wide)

    # ---- rel runs: per (tile, type) cap, 32-aligned, packed into 128-bins ----
    cntR = np.zeros((NC, T, NREL), np.int64)
    np.add.at(cntR, (core_r, t_r, rtype), 1)
    capR = cntR.max(axis=0)                        # [T, NREL]
    bins = []      # per tile: list of bins; bin = list of (r, off, size32)
    runloc = np.zeros((T, NREL, 3), np.int64)      # bin slot id, off, size32 per run
    runloc[:, :, 0] = -1
    nbinsR = np.zeros(T, np.int64)
    binbaseR = np.zeros(T + 1, np.int64)
    for t in range(T):
        sizes = [(int(min(_ceil(capR[t, r], 32) * 32, P)), r)
                 for r in range(NREL) if capR[t, r] > 0]
        # runs longer than 128 must split into multiple runs of the same type
        expanded = []
        for sz, r in sizes:
            rem = int(capR[t, r])
            while rem > 0:
                take = min(rem, P)
                expanded.append((min(_ceil(take, 32) * 32, P), r, take))
                rem -= take
        expanded.sort(reverse=True)
        tbins = []
        for sz, r, take in expanded:
            placed = False
            for b in tbins:
                used = sum(x[2] for x in b)
                if used + sz <= P:
                    b.append((r, used, sz, take))
                    placed = True
                    break
            if not placed:
                tbins.append([(r, 0, sz, take)])
        bins.append(tbins)
        nbinsR[t] = len(tbins)
        binbaseR[t + 1] = binbaseR[t] + len(tbins)
    SR = int(binbaseR[-1])                         # total rel slots

    # run placement map for scattering edges: (t, r, k-th split) -> slot, off
    # rebuild an ordered per (t, r) list of (slot, off, take)
    runsplits = {}
    for t in range(T):
        for bi, b in enumerate(bins[t]):
            for (r, off, sz, take) in b:
                runsplits.setdefault((t, r), []).append(
                    (int(binbaseR[t] + bi), off, take))

    # ---- granules (gather calls), grouped per supertile ----
    GMAX = 16
    granA, granR = [], []     # per supertile: list of (slot_start, nslots)
    sA_lo = [int(slotA_base[st_lo[s]]) for s in range(S)]
    sA_hi = [int(slotA_base[st_hi[s]]) for s in range(S)]
    sR_lo = [int(binbaseR[st_lo[s]]) for s in range(S)]
    sR_hi = [int(binbaseR[st_hi[s]]) for s in range(S)]
    for s in range(S):
        ga = []
        k = sA_lo[s]
        while k < sA_hi[s]:
            n = min(GMAX, sA_hi[s] - k)
            ga.append((k, n))
            k += n
        granA.append(ga)
        gr = []
        k = sR_lo[s]
        while k < sR_hi[s]:
            n = min(GMAX, sR_hi[s] - k)
            gr.append((k, n))
            k += n
        granR.append(gr)

    # ================= per-core data =================
    # edge placement position within streams
    def place(core, tt, key_counts_shape, base_of, which):
        # returns per-edge stream position for this core
        pass

    # attr positions
    posA = np.full(d["E_ATTR"], -1, np.int64)
    for c in range(NC):
        m = core_a == c
        tt = t_a[m]
        order = np.argsort(tt, kind="stable")
        idxs = np.nonzero(m)[0][order]
        tt_sorted = tt[order]
        # rank within tile
        start = np.searchsorted(tt_sorted, np.arange(T))
        ranks = np.arange(len(tt_sorted)) - start[tt_sorted]
        posA[idxs] = slotA_base[tt_sorted] * P + ranks
    # rel positions
    posR = np.full(d["E_REL"], -1, np.int64)
    for c in range(NC):
        m = core_r == c
        key = t_r[m] * NREL + rtype[m]
        order = np.argsort(key, kind="stable")
        idxs = np.nonzero(m)[0][order]
        key_sorted = key[order]
        start = np.searchsorted(key_sorted, np.arange(T * NREL))
        ranks = np.arange(len(key_sorted)) - start[key_sorted]
        # map (t, r, rank) -> position via runsplits
        tt = key_sorted // NREL
        rr = key_sorted % NREL
        pos = np.empty(len(idxs), np.int64)
        for (t, r), splits in runsplits.items():
            mm = (tt == t) & (rr == r)
            if not mm.any():
                continue
            rk = ranks[mm]
            p = np.empty(len(rk), np.int64)
            lo = 0
            for (slot, off, take) in splits:
                sel = (rk >= lo) & (rk < lo + take)
                p[sel] = slot * P + off + (rk[sel] - lo)
                lo += take
            pos[mm] = p
        posR[idxs] = pos

    maps = []
    plan_tab = None
    for c in range(NC):
        mA = core_a == c
        mR = core_r == c
        pA = posA[mA]
        pR = posR[mR]

        dstoffA = np.full(SA * P, -1.0, np.float32)
        scaleA = np.zeros(SA * P, np.float32)
        srcA_g = np.zeros(SA * P, np.int64)
        realA = np.zeros(SA * P, bool)
        dstoffA[pA] = off_a[mA]
        scaleA[pA] = sca_e[dst_a[mA]]
        srcA_g[pA] = src_a[mA]
        realA[pA] = True
        eT = np.zeros((d["EDGE_DIM"] + 1, SA * P), np.float32)
        eT[:d["EDGE_DIM"], pA] = edge_emb[mA].T
        eT[d["EDGE_DIM"], :] = 1.0

        dstoffR = np.full(SR * P, -1.0, np.float32)
        scaleR = np.zeros(SR * P, np.float32)
        srcR_g = np.zeros(SR * P, np.int64)
        realR = np.zeros(SR * P, bool)
        dstoffR[pR] = off_r[mR]
        scaleR[pR] = scr_e[dst_r[mR] * NREL + rtype[mR]]
        srcR_g[pR] = src_r[mR]
        realR[pR] = True

        # ---- local tables per supertile ----
        uniqs, locA, locR = [], np.zeros(SA * P, np.int64), np.zeros(SR * P, np.int64)
        for s in range(S):
            aa = slice(sA_lo[s] * P, sA_hi[s] * P)
            rr_ = slice(sR_lo[s] * P, sR_hi[s] * P)
            srcs = np.concatenate([srcA_g[aa][realA[aa]], srcR_g[rr_][realR[rr_]]])
            uniq = np.unique(srcs) if len(srcs) else np.zeros(1, np.int64)
            uniqs.append(uniq)
            locA[aa] = np.where(realA[aa], np.searchsorted(uniq, srcA_g[aa]), 0)
            locR[rr_] = np.where(realR[rr_], np.searchsorted(uniq, srcR_g[rr_]), 0)
        maps.append(dict(
            dstoffA=dstoffA, scaleA=scaleA, eT=eT, locA=locA,
            dstoffR=dstoffR, scaleR=scaleR, locR=locR, uniqs=uniqs,
        ))

    cap_s = [max(len(maps[c]["uniqs"][s]) for c in range(NC)) for s in range(S)]
    tab_base = np.concatenate([[0], np.cumsum(cap_s)]).astype(np.int64)
    TAB = int(tab_base[-1])

    # ---- finalize per-core tensors ----
    def wrap_idx(flat):
        n = len(flat)
        w = np.zeros((16, n // 16), np.int16)
        w[np.arange(n) % 16, np.arange(n) // 16] = flat.astype(np.int16)
        return np.tile(w, (8, 1))

    in_maps = []
    for c in range(NC):
        m = maps[c]
        tabc = np.zeros((max(TAB, 1), d["NODE_DIM"]), np.float32)
        for s in range(S):
            u = m["uniqs"][s]
            tabc[tab_base[s]:tab_base[s] + len(u)] = node_emb[u]
        idxA = np.concatenate([
            wrap_idx(m["locA"][k0 * P:(k0 + n) * P] - 0)
            for s in range(S) for (k0, n) in granA[s]], axis=1) if SA else np.zeros((128, 1), np.int16)
        idxR = np.concatenate([
            wrap_idx(m["locR"][k0 * P:(k0 + n) * P] - 0)
            for s in range(S) for (k0, n) in granR[s]], axis=1) if SR else np.zeros((128, 1), np.int16)

        n0, n1 = c * NPC, (c + 1) * NPC
        node_own = np.zeros((NPC_PAD, d["NODE_DIM"]), np.float32)
        node_own[:NPC] = node_emb[n0:n1]
        isu = np.zeros((P, T), np.uint8)
        iu = is_unit[n0:n1].astype(np.uint8)
        iu = np.concatenate([iu, np.zeros(NPC_PAD - NPC, np.uint8)])
        isu[:, :] = iu.reshape(T, P).T

        W_msg = np.asarray(inputs["W_msg"], np.float32)
        b_msg = np.asarray(inputs["b_msg"], np.float32)
        w_e_aug = np.concatenate([W_msg[d["NODE_DIM"]:], b_msg[None, :]], axis=0)
        W_rel = np.asarray(inputs["W_rel"], np.float32)      # [8,128,128]
        wrel = np.transpose(W_rel, (1, 0, 2)).reshape(d["NODE_DIM"], -1)  # [128, 8*128]
        W_unit = np.asarray(inputs["W_unit"], np.float32)    # [384,128]
        W_attr = np.asarray(inputs["W_attr"], np.float32)    # [256,128]
        wu = W_unit.reshape(3, P, d["OUT_DIM"]).transpose(1, 0, 2).reshape(P, -1)
        wa = W_attr.reshape(2, P, d["OUT_DIM"]).transpose(1, 0, 2).reshape(P, -1)
        iota = np.tile(np.arange(P, dtype=np.float32), (P, 1))

        in_maps.append(dict(
            tab=tabc, idx_a=idxA, idx_r=idxR,
            dstoff_a=m["dstoffA"].reshape(SA, P).T.copy() if SA else np.zeros((P, 1), np.float32),
            scale_a=m["scaleA"].reshape(SA, P).T.copy() if SA else np.zeros((P, 1), np.float32),
            dstoff_r=m["dstoffR"].reshape(SR, P).T.copy() if SR else np.zeros((P, 1), np.float32),
            scale_r=m["scaleR"].reshape(SR, P).T.copy() if SR else np.zeros((P, 1), np.float32),
            e_t=m["eT"], node_own=node_own, is_unit_f=isu,
            w_x=W_msg[:d["NODE_DIM"]].copy(), w_e=w_e_aug, w_rel=wrel,
            w_unit=wu, w_attr=wa, iota=iota,
        ))

    # biases: b_msg folded via eT ones row; others must be zero (they are in
    # this problem); verify so we never return silently wrong results.
    for k in ("b_rel", "b_unit", "b_attr"):
        assert not np.any(np.asarray(inputs[k])), f"{k} nonzero: not supported"

    plan = dict(
        T=T, S=S, ST=ST, NPC_PAD=NPC_PAD, SA=SA, SR=SR, TAB=TAB,
        chunksA=chunksA.tolist(), slotA_base=slotA_base.tolist(),
        bins=bins, binbaseR=binbaseR.tolist(),
        granA=granA, granR=granR,
        sA_lo=sA_lo, sA_hi=sA_hi, sR_lo=sR_lo, sR_hi=sR_hi,
        st_lo=st_lo, st_hi=st_hi,
        cap_s=cap_s, tab_base=tab_base.tolist(),
    )
    return plan, in_maps


def build_nc(plan, dims, ncores):
    d = dims
    f32, i16 = mybir.dt.float32, mybir.dt.int16
    T, S = plan["T"], plan["S"]
    SA, SR, TAB = plan["SA"], plan["SR"], plan["TAB"]
    ED1 = d["EDGE_DIM"] + 1
    NPC_PAD = plan["NPC_PAD"]

    nc = bacc.Bacc("TRN2", target_bir_lowering=False, debug=False,
                   num_devices=ncores)

    def din(name, shape):
        return nc.dram_tensor(name, shape, f32, kind="ExternalInput").ap()

    tab = din("tab", [max(TAB, 1), d["NODE_DIM"]])
    idx_a = nc.dram_tensor("idx_a", [P, max(SA * 8, 1)], i16, kind="ExternalInput").ap()
    idx_r = nc.dram_tensor("idx_r", [P, max(SR * 8, 1)], i16, kind="ExternalInput").ap()
    dstoff_a = din("dstoff_a", [P, max(SA, 1)])
    scale_a = din("scale_a", [P, max(SA, 1)])
    dstoff_r = din("dstoff_r", [P, max(SR, 1)])
    scale_r = din("scale_r", [P, max(SR, 1)])
    e_t = din("e_t", [ED1, max(SA * P, 1)])
    node_own = din("node_own", [NPC_PAD, d["NODE_DIM"]])
    is_unit_f = nc.dram_tensor("is_unit_f", [P, T], mybir.dt.uint8, kind="ExternalInput").ap()
    w_x = din("w_x", [d["NODE_DIM"], d["MSG_DIM"]])
    w_e = din("w_e", [ED1, d["MSG_DIM"]])
    w_rel = din("w_rel", [d["NODE_DIM"], d["N_REL"] * d["MSG_DIM"]])
    w_unit = din("w_unit", [P, 3 * d["OUT_DIM"]])
    w_attr = din("w_attr", [P, 2 * d["OUT_DIM"]])
    iota = din("iota", [P, P])
    out = nc.dram_tensor("out", [NPC_PAD, d["OUT_DIM"]], f32,
                         kind="ExternalOutput").ap()

    Relu = mybir.ActivationFunctionType.Relu

    with tile.TileContext(nc) as tc, ExitStack() as ctx:
        const = ctx.enter_context(tc.tile_pool(name="const", bufs=1))
        stream = ctx.enter_context(tc.tile_pool(name="stream", bufs=2))
        gxp = ctx.enter_context(tc.tile_pool(name="gx", bufs=3))
        work = ctx.enter_context(tc.tile_pool(name="work", bufs=3))
        hseg = ctx.enter_context(tc.tile_pool(name="hseg", bufs=2))
        psw = ctx.enter_context(tc.tile_pool(name="psw", bufs=2, space="PSUM"))
        pshu = ctx.enter_context(tc.tile_pool(name="pshu", bufs=1, space="PSUM"))
        pseg = ctx.enter_context(tc.tile_pool(name="pseg", bufs=1, space="PSUM"))

        nc.gpsimd.load_library(library_config.mlp)

        ident = const.tile([P, P], f32)
        make_identity(nc, ident[:])
        iota_sb = const.tile([P, P], f32)
        nc.sync.dma_start(out=iota_sb[:], in_=iota[:])
        wx_sb = const.tile([d["NODE_DIM"], d["MSG_DIM"]], f32)
        nc.sync.dma_start(out=wx_sb[:], in_=w_x[:])
        we_sb = const.tile([ED1, d["MSG_DIM"]], f32)
        nc.sync.dma_start(out=we_sb[:], in_=w_e[:])
        wr_sb = const.tile([d["NODE_DIM"], d["N_REL"] * d["MSG_DIM"]], f32)
        nc.sync.dma_start(out=wr_sb[:], in_=w_rel[:])
        wu_sb = const.tile([P, 3 * d["OUT_DIM"]], f32)
        nc.sync.dma_start(out=wu_sb[:], in_=w_unit[:])
        wa_sb = const.tile([P, 2 * d["OUT_DIM"]], f32)
        nc.sync.dma_start(out=wa_sb[:], in_=w_attr[:])
        isu_sb = const.tile([P, T], mybir.dt.uint8)
        nc.sync.dma_start(out=isu_sb[:], in_=is_unit_f[:])

        # per-supertile idx stream col offsets
        gcolA, gcolR = [], []
        ca = cr = 0
        for s in range(S):
            ga, gr = [], []
            for (k0, n) in plan["granA"][s]:
                ga.append((ca, n))
                ca += n * 8
            for (k0, n) in plan["granR"][s]:
                gr.append((cr, n))
                cr += n * 8
            gcolA.append(ga)
            gcolR.append(gr)

        nalt = [0]  # alternator for ACT/DVE balancing

        def relu_scale(dst, src_ps, scale_ap):
            if nalt[0] % 2 == 0:
                nc.scalar.activation(out=dst, in_=src_ps, func=Relu,
                                     scale=scale_ap)
            else:
                nc.vector.tensor_scalar(
                    out=dst, in0=src_ps, scalar1=scale_ap, scalar2=0.0,
                    op0=mybir.AluOpType.mult, op1=mybir.AluOpType.max)
            nalt[0] += 1

        def copy_ps(dst, src_ps):
            if nalt[0] % 2 == 0:
                nc.scalar.copy(out=dst, in_=src_ps)
            else:
                nc.vector.tensor_copy(out=dst, in_=src_ps)
            nalt[0] += 1

        for s in range(S):
            a0, a1 = plan["sA_lo"][s], plan["sA_hi"][s]
            r0, r1 = plan["sR_lo"][s], plan["sR_hi"][s]
            na, nr = a1 - a0, r1 - r0
            tlo, thi = plan["st_lo"][s], plan["st_hi"][s]
            ntl = thi - tlo

            # stream loads for this supertile
            doa = stream.tile([P, max(na, 1)], f32, tag="doa")
            sca = stream.tile([P, max(na, 1)], f32, tag="sca")
            dor = stream.tile([P, max(nr, 1)], f32, tag="dor")
            scr = stream.tile([P, max(nr, 1)], f32, tag="scr")
            ets = stream.tile([ED1, max(na, 1) * P], f32, tag="ets")
            xns = stream.tile([P, ntl, d["NODE_DIM"]], f32, tag="xns")
            if na:
                nc.sync.dma_start(out=doa[:], in_=dstoff_a[:, a0:a1])
                nc.sync.dma_start(out=sca[:], in_=scale_a[:, a0:a1])
                nc.sync.dma_start(out=ets[:], in_=e_t[:, a0 * P:a1 * P])
            if nr:
                nc.sync.dma_start(out=dor[:], in_=dstoff_r[:, r0:r1])
                nc.sync.dma_start(out=scr[:], in_=scale_r[:, r0:r1])
            nc.sync.dma_start(
                out=xns[:],
                in_=node_own[tlo * P:thi * P, :].rearrange(
                    "(t p) d -> p t d", p=P))

            tb, cs = plan["tab_base"][s], plan["cap_s"][s]
            tabs = tab[tb:tb + cs, :]

            # idx loads + gathers for this supertile
            gxa, gxr = [], []
            if na:
                ia = stream.tile([P, max(8 * na, 1)], i16, tag="ia")
                nc.sync.dma_start(
                    out=ia[:], in_=idx_a[:, gcolA[s][0][0]:gcolA[s][0][0] + 8 * na])
            if nr:
                ir = stream.tile([P, max(8 * nr, 1)], i16, tag="ir")
                nc.sync.dma_start(
                    out=ir[:], in_=idx_r[:, gcolR[s][0][0]:gcolR[s][0][0] + 8 * nr])
            for gi, (k0, n) in enumerate(plan["granA"][s]):
                gx = gxp.tile([P, 16, d["NODE_DIM"]], f32, tag="gxa")
                c0 = gcolA[s][gi][0] - gcolA[s][0][0]
                nc.gpsimd.dma_gather(
                    out_ap=gx[:, :n, :], in_ap=tabs, idxs_ap=ia[:, c0:c0 + 8 * n],
                    num_idxs=n * P, num_idxs_reg=n * P,
                    elem_size=d["NODE_DIM"], single_packet=False)
                gxa.append((k0, n, gx))
            for gi, (k0, n) in enumerate(plan["granR"][s]):
                gx = gxp.tile([P, 16, d["NODE_DIM"]], f32, tag="gxr")
                c0 = gcolR[s][gi][0] - gcolR[s][0][0]
                nc.gpsimd.dma_gather(
                    out_ap=gx[:, :n, :], in_ap=tabs, idxs_ap=ir[:, c0:c0 + 8 * n],
                    num_idxs=n * P, num_idxs_reg=n * P,
                    elem_size=d["NODE_DIM"], single_packet=False)
                gxr.append((k0, n, gx))

            def slotA(k):
                for (k0, n, gx) in gxa:
                    if k0 <= k < k0 + n:
                        return gx[:, k - k0, :]
                raise KeyError(k)

            def slotR(k):
                for (k0, n, gx) in gxr:
                    if k0 <= k < k0 + n:
                        return gx[:, k - k0, :]
                raise KeyError(k)

            for t in range(tlo, thi):
                segA = pseg.tile([P, P], f32, tag="segA", space="PSUM")
                segR = pseg.tile([P, P], f32, tag="segR", space="PSUM")

                # ---- attribute message chunks ----
                ka0 = plan["slotA_base"][t]
                nka = plan["chunksA"][t]
                for j in range(nka):
                    k = ka0 + j
                    X = slotA(k)
                    xt_ps = psw.tile([P, P], f32, tag="xt", space="PSUM")
                    nc.tensor.transpose(out=xt_ps[:], in_=X, identity=ident[:])
                    xt = work.tile([P, P], f32, tag="xt_sb")
                    copy_ps(xt[:], xt_ps[:])
                    m_ps = psw.tile([P, P], f32, tag="m", space="PSUM")
                    nc.tensor.matmul(out=m_ps[:], lhsT=xt[:], rhs=wx_sb[:],
                                     start=True, stop=False, skip_group_check=True)
                    nc.tensor.matmul(out=m_ps[:],
                                     lhsT=ets[:, (k - a0) * P:(k - a0 + 1) * P],
                                     rhs=we_sb[:],
                                     start=False, stop=True, skip_group_check=True)
                    msb = work.tile([P, P], f32, tag="m_sb")
                    relu_scale(msb[:], m_ps[:], sca[:, k - a0:k - a0 + 1])
                    oh = work.tile([P, P], f32, tag="oh")
                    nc.vector.tensor_tensor(
                        out=oh[:], in0=doa[:, k - a0:k - a0 + 1].to_broadcast([P, P]),
                        in1=iota_sb[:], op=mybir.AluOpType.is_equal)
                    nc.tensor.matmul(out=segA[:], lhsT=msb[:], rhs=oh[:],
                                     start=(j == 0), stop=(j == nka - 1),
                                     skip_group_check=True)

                # ---- relational message bins ----
                tbins = plan["bins"][t]
                nkb = len(tbins)
                for j in range(nkb):
                    k = plan["binbaseR"][t] + j
                    X = slotR(k)
                    xt_ps = psw.tile([P, P], f32, tag="xt", space="PSUM")
                    nc.tensor.transpose(out=xt_ps[:], in_=X, identity=ident[:])
                    xt = work.tile([P, P], f32, tag="xtr_sb")
                    copy_ps(xt[:], xt_ps[:])
                    m_ps = psw.tile([P, P], f32, tag="m", space="PSUM")
                    for (r, off, sz, take) in tbins[j]:
                        nc.tensor.matmul(
                            out=m_ps[off:off + sz, :],
                            lhsT=xt[:, off:off + sz],
                            rhs=wr_sb[:, r * P:(r + 1) * P],
                            start=True, stop=True, skip_group_check=True,
                            tile_position=(0, off))
                    msb = work.tile([P, P], f32, tag="mr_sb")
                    relu_scale(msb[:], m_ps[:], scr[:, k - r0:k - r0 + 1])
                    oh = work.tile([P, P], f32, tag="ohr")
                    nc.vector.tensor_tensor(
                        out=oh[:], in0=dor[:, k - r0:k - r0 + 1].to_broadcast([P, P]),
                        in1=iota_sb[:], op=mybir.AluOpType.is_equal)
                    nc.tensor.matmul(out=segR[:], lhsT=msb[:], rhs=oh[:],
                                     start=(j == 0), stop=(j == nkb - 1),
                                     skip_group_check=True)
                if nkb == 0:
                    nc.vector.memset(segR[:], 0.0)

                # ---- node update ----
                xn = xns[:, t - tlo, :]
                xnt_ps = psw.tile([P, P], f32, tag="xt", space="PSUM")
                nc.tensor.transpose(out=xnt_ps[:], in_=xn, identity=ident[:])
                xnt = work.tile([P, P], f32, tag="xnt_sb")
                copy_ps(xnt[:], xnt_ps[:])
                attrT = hseg.tile([P, P], f32, tag="attrT")
                copy_ps(attrT[:], segA[:])
                relT = hseg.tile([P, P], f32, tag="relT")
                copy_ps(relT[:], segR[:])

                hu_ps = pshu.tile([P, P], f32, tag="hu", space="PSUM")
                nc.tensor.matmul(out=hu_ps[:], lhsT=xnt[:], rhs=wu_sb[:, 0:P],
                                 start=True, stop=False, skip_group_check=True)
                nc.tensor.matmul(out=hu_ps[:], lhsT=attrT[:], rhs=wu_sb[:, P:2 * P],
                                 start=False, stop=False, skip_group_check=True)
                nc.tensor.matmul(out=hu_ps[:], lhsT=relT[:], rhs=wu_sb[:, 2 * P:3 * P],
                                 start=False, stop=True, skip_group_check=True)
                ha_ps = pshu.tile([P, P], f32, tag="ha", space="PSUM")
                nc.tensor.matmul(out=ha_ps[:], lhsT=xnt[:], rhs=wa_sb[:, 0:P],
                                 start=True, stop=False, skip_group_check=True)
                nc.tensor.matmul(out=ha_ps[:], lhsT=attrT[:], rhs=wa_sb[:, P:2 * P],
                                 start=False, stop=True, skip_group_check=True)

                hout = work.tile([P, P], f32, tag="hout")
                nc.scalar.activation(out=hout[:], in_=ha_ps[:], func=Relu)
                hu = work.tile([P, P], f32, tag="husb")
                nc.vector.tensor_scalar(
                    out=hu[:], in0=hu_ps[:], scalar1=1.0, scalar2=0.0,
                    op0=mybir.AluOpType.mult, op1=mybir.AluOpType.max)
                nc.vector.copy_predicated(
                    out=hout[:], mask=isu_sb[:, t:t + 1].to_broadcast([P, P]),
                    data=hu[:])
                nc.sync.dma_start(out=out[t * P:(t + 1) * P, :], in_=hout[:])

    nc.compile()
    return nc


_CACHE = {}


def kernel(**inputs) -> np.ndarray:
    dims = FULL_DIMS
    plan, in_maps = prep(inputs, dims)
    key = "full"
    if key not in _CACHE:
        _CACHE[key] = build_nc(plan, dims, dims["NCORES"])
    nc = _CACHE[key]
    res = run_bass_kernel_spmd(nc, in_maps, list(range(dims["NCORES"])))
    outs = [np.asarray(res.results[c]["out"])[:dims["NPC"]]
            for c in range(dims["NCORES"])]
    return np.concatenate(outs, axis=0)
